# revision 15
# baseline (speedup 1.0000x reference)
"""AudioEncoder Trainium2 kernel.

Computes: conv1d(1->64, k=5, stride=2, pad=2) + bias -> ReLU -> per-timestep
linear (64->64) + bias, over audio [4, 480000] f32 -> out [4, 240000, 64] f32.

Strategy (pure data parallel over 8 cores):
  - Each core handles one half of one batch row, padded to S_PAD = 59*2048 =
    120832 output positions (positions >= 120000 are junk, discarded on host).
  - Host pre-gathers the 5 conv tap streams (tap k of output position j is
    x[2j + k - 2], zero-padded) and PERMUTES each into the on-chip column
    order, so every on-chip access is contiguous:
      within each 2048-position block, column c = g*1024 + bi*128 + r
      (g=c//1024, bi=(c//128)%8, r=c%128) holds position u + 16r + 8g + bi.
  - The permuted [5, S_PAD] tap matrix is DMAed twice per 8192-position
    super-block: to SBUF partitions 0-4 (PE row group 0) and 32-36 (row
    group 1), so the four conv matmuls of a chunk run on four disjoint PE
    quadrants (row groups 0/32 x col groups 0/64) with contiguous fp16
    moving operands:
      psc[ 0: 64,   0: 512] <- rg0 cols [0,512)     (g=0, bi=0-3)
      psc[ 0: 64, 512:1024] <- rg1 cols [512,1024)  (g=0, bi=4-7)
      psc[64:128,   0: 512] <- rg0 cols [1024,1536) (g=1, bi=0-3)
      psc[64:128, 512:1024] <- rg1 cols [1536,2048) (g=1, bi=4-7)
    One [128, 1024] PSUM tile (2 banks) holds feats for 2048 positions:
    feats[64g + e, bi*128 + r] = channel e of position u + 16r + 8g + bi.
  - ACT applies conv bias + ReLU in ONE [128, 1024] op (PSUM -> SBUF fp16).
  - Linear: 16 fp16 matmuls per chunk; feats 128-col blocks stationary,
    lin_w.T moving.  A (feats rows 0-63) and B (rows 64-127) matmuls run
    concurrently in different PE row groups and write the two separate PSUM
    banks of one [128, 1024] tile: psl[r, (8g + bi)*64 + p] = out position
    u + 16r + (8g + bi), feature p.  (Same-partition same-bank concurrent
    writes are a HW fault; different banks are safe.)
  - DVE adds the (pre-broadcast) linear bias in ONE [128, 1024] op, giving
    outt[r, t*64 + p] = out(u + 16r + t, p) -- so each SBUF partition is ONE
    4 KiB contiguous DRAM run: the store DMA is 128 descriptors x 4 KiB.
  - PSUM budget: conv 2x[128,1024] + linear 2x[128,1024] double-buffered
    = exactly 8 banks.
"""

import numpy as np

import concourse.bacc as bacc
import concourse.bass as bass
import concourse.mybir as mybir
import concourse.tile as tile
from concourse.bass_utils import run_bass_kernel_spmd

B = 4
T = 480000
S_FULL = 240000  # conv output positions per batch row
N_CORES = 8
S_CORE = S_FULL * B // N_CORES  # 120000 positions per core
CHUNK = 2048  # output positions per chunk
S_PAD = ((S_CORE + CHUNK - 1) // CHUNK) * CHUNK  # 120832
SUPER = 8192  # output positions covered per im2col load
E = 64  # conv out channels
P = 64  # linear out features
KS = 5

f16 = mybir.dt.float16
f32 = mybir.dt.float32


def emit(nc: bass.Bass, S: int = S_PAD) -> None:
    """Emit the per-core Tile kernel for S (chunk-aligned) output positions."""
    from contextlib import ExitStack

    im_d = nc.declare_dram_parameter("im5", [KS, S], f16, isOutput=False)
    wc_d = nc.declare_dram_parameter("wc", [40, E], f16, isOutput=False)
    cb_d = nc.declare_dram_parameter("cb", [128, 1], f32, isOutput=False)
    w2_d = nc.declare_dram_parameter("w2", [128, P], f16, isOutput=False)
    b2_d = nc.declare_dram_parameter("b2", [128, 16 * P], f32, isOutput=False)
    out_d = nc.declare_dram_parameter("out", [S, P], f32, isOutput=True)

    RELU = mybir.ActivationFunctionType.Relu
    HALF = CHUNK // 2  # 1024
    assert S % CHUNK == 0

    with tile.TileContext(nc) as tc, ExitStack() as ctx:
        consts = ctx.enter_context(tc.tile_pool(name="consts", bufs=1))
        imp = ctx.enter_context(tc.tile_pool(name="im", bufs=3))
        fpool = ctx.enter_context(tc.tile_pool(name="feats", bufs=3))
        opool = ctx.enter_context(tc.tile_pool(name="outs", bufs=3))
        pc = ctx.enter_context(tc.tile_pool(name="psc", bufs=2, space="PSUM"))
        pl = ctx.enter_context(tc.tile_pool(name="psl", bufs=2, space="PSUM"))

        def load_super(sbase, eng):
            """Load the permuted tap matrix, twice: partitions 0-4 (PE row
            group 0) and 32-36 (row group 1)."""
            scount = min(SUPER, S - sbase)
            t = imp.tile([37, SUPER], f16)
            eng.dma_start(
                out=t[0:KS, 0:scount], in_=im_d[:, sbase : sbase + scount]
            )
            eng.dma_start(
                out=t[32 : 32 + KS, 0:scount],
                in_=im_d[:, sbase : sbase + scount],
            )
            return t

        # super 0 goes FIRST on the sync HWDGE ring so nothing delays the
        # first chunk; the consts queue behind it, then super 1 (still on
        # the sync ring -- its buffer is free, so it can never
        # head-of-line-block the stores).  Later supers are emitted one
        # super AHEAD of use on the gpsimd SWDGE ring: the SWDGE issue
        # waits only on that buffer's release (~4 chunks early with
        # bufs=3), so the data always arrives before the chunk needs it,
        # and a waiting load blocks nothing.
        n_super = (S + SUPER - 1) // SUPER
        tiles = {0: load_super(0, nc.sync)}

        wc_sb = consts.tile([40, E], f16)
        nc.sync.dma_start(out=wc_sb[:, :], in_=wc_d[:, :])
        cb_sb = consts.tile([128, 1], f32)
        nc.sync.dma_start(out=cb_sb[:, :], in_=cb_d[:, :])
        w2_sb = consts.tile([128, P], f16)
        nc.sync.dma_start(out=w2_sb[:, :], in_=w2_d[:, :])
        b2_sb = consts.tile([128, 16 * P], f32)
        nc.sync.dma_start(out=b2_sb[:, :], in_=b2_d[:, :])

        if n_super > 1:
            tiles[1] = load_super(SUPER, nc.sync)

        for b in range(0, S, CHUNK):
            si = b // SUPER
            im = tiles[si]

            c0 = b - si * SUPER
            # conv: 4 concurrent matmuls on disjoint PE quadrants, all with
            # contiguous [5, 512] fp16 moving operands.
            psc = pc.tile([128, HALF], f32)  # 2 banks
            for g in (0, 1):  # psum partition half
                q = c0 + 1024 * g
                nc.tensor.matmul(
                    out=psc[E * g : E * g + E, 0:512],
                    lhsT=wc_sb[0:KS, :],
                    rhs=im[0:KS, q : q + 512],
                    start=True, stop=True,
                )
                nc.tensor.matmul(
                    out=psc[E * g : E * g + E, 512:1024],
                    lhsT=wc_sb[32 : 32 + KS, :],
                    rhs=im[32 : 32 + KS, q + 512 : q + 1024],
                    start=True, stop=True,
                )

            feats = fpool.tile([128, HALF], f16)
            nc.scalar.activation(
                out=feats[:, :], in_=psc[:, :], func=RELU,
                bias=cb_sb[:, 0:1], scale=1.0,
            )

            # linear: 8 A/B concurrent pairs; A -> bank 0, B -> bank 1.
            psl = pl.tile([128, HALF], f32)  # 2 banks
            for bi in range(8):
                nc.tensor.matmul(
                    out=psl[:, P * bi : P * bi + P],
                    lhsT=feats[0:E, 128 * bi : 128 * bi + 128],
                    rhs=w2_sb[0:E, :], start=True, stop=True,
                )
                nc.tensor.matmul(
                    out=psl[:, 512 + P * bi : 512 + P * bi + P],
                    lhsT=feats[E : 2 * E, 128 * bi : 128 * bi + 128],
                    rhs=w2_sb[E : 2 * E, :], start=True, stop=True,
                )

            outt = opool.tile([128, HALF], f32)
            nc.vector.tensor_add(outt[:, :], psl[:, :], b2_sb[:, :])

            # s = b + 16r + t ; sbuf col = t*64 + p  (4 KiB run per partition)
            dview = out_d[b : b + CHUNK, :].rearrange("(r t) p -> r t p", t=16)
            sview = outt[:, :].rearrange("r (t p) -> r t p", t=16)
            nc.sync.dma_start(out=dview, in_=sview)

            # prefetch super si+2 right after the first store of super si.
            # Supers 2-3 ride the sync ring: the engine only reaches them
            # after this chunk's store issue, so they cannot contend with
            # the critical super-0/1 loads at t=0 (their buffers are free,
            # so they never head-of-line-block later stores).  Supers >= 4
            # go on the gpsimd ring, where the im pool's slot recycling
            # (bufs=3) gates them to ~8 chunks before first use.
            ci = b // CHUNK
            if ci % 4 == 0 and ci // 4 + 2 < n_super:
                k = ci // 4 + 2
                tiles[k] = load_super(
                    k * SUPER, nc.sync if k <= 3 else nc.gpsimd
                )
                tiles.pop(k - 3, None)


def prep_shared(conv_w, conv_b, lin_w, lin_b):
    """Host-side prep of the (tiny, replicated) parameter tensors."""
    conv_w = np.asarray(conv_w, dtype=np.float32)
    conv_b = np.asarray(conv_b, dtype=np.float32)
    lin_w = np.asarray(lin_w, dtype=np.float32)
    lin_b = np.asarray(lin_b, dtype=np.float32)

    wc5 = conv_w[:, 0, :].T.astype(np.float16)  # [5 taps, 64]
    wc = np.zeros((40, E), dtype=np.float16)
    wc[0:5] = wc5
    wc[32:37] = wc5
    cb = np.ascontiguousarray(
        np.concatenate([conv_b, conv_b]).astype(np.float32)[:, None]
    )  # [128, 1]
    w2 = lin_w.T.astype(np.float16)  # [64e, 64p]
    w2s = np.ascontiguousarray(np.concatenate([w2, w2], axis=0))  # [128, 64]
    b2 = np.ascontiguousarray(
        np.tile(lin_b.astype(np.float32)[None, :], (128, 16))
    )  # [128, 1024]
    return wc, cb, w2s, b2


def prep_inputs(audio_waveform, conv_w, conv_b, lin_w, lin_b):
    """Host-side shard + dtype/layout prep. Returns in_maps for the 8 cores."""
    x = np.asarray(audio_waveform, dtype=np.float32)
    assert x.shape == (B, T)
    # xp[b, 2 + t] = x[b, t]; wide enough for every core's padded window.
    width = 2 * (S_FULL - S_CORE) + 2 * S_PAD + 4
    width = max(width, T + 4)
    xp = np.zeros((B, width), dtype=np.float16)
    xp[:, 2 : 2 + T] = x.astype(np.float16)

    wc, cb, w2s, b2 = prep_shared(conv_w, conv_b, lin_w, lin_b)

    in_maps = []
    for c in range(N_CORES):
        b_i, h = divmod(c, 2)
        s0 = h * S_CORE
        xc = xp[b_i, 2 * s0 : 2 * s0 + 2 * S_PAD + 4]
        # tap k stream (position j -> x[2j + k - 2]), permuted per 2048-block
        # into column order c = g*1024 + bi*128 + r <-> position 16r + 8g + bi.
        rows = np.empty((KS, S_PAD), dtype=np.float16)
        for k in range(KS):
            tap = xc[k : k + 2 * S_PAD : 2]  # [S_PAD]
            rows[k] = (
                tap.reshape(-1, 128, 2, 8).transpose(0, 2, 3, 1).reshape(S_PAD)
            )
        in_maps.append(
            dict(im5=np.ascontiguousarray(rows), wc=wc, cb=cb, w2=w2s, b2=b2)
        )
    return in_maps


_NC_CACHE = None


def get_nc() -> bass.Bass:
    global _NC_CACHE
    if _NC_CACHE is None:
        nc = bacc.Bacc()
        emit(nc)
        # Legalizes TRN2 sync constraints (splits multi-wait instructions),
        # allocates registers, etc. Required before walrus codegen.
        nc.compile()
        _NC_CACHE = nc
    return _NC_CACHE


def run(inputs: dict, trace: bool = False):
    """Run on the 8 cores; returns (full_output, BassKernelResults)."""
    in_maps = prep_inputs(**inputs)
    nc = get_nc()
    res = run_bass_kernel_spmd(nc, in_maps, list(range(N_CORES)), trace=trace)
    out = np.empty((B, S_FULL, P), dtype=np.float32)
    for c in range(N_CORES):
        b_i, h = divmod(c, 2)
        out[b_i, h * S_CORE : (h + 1) * S_CORE, :] = res.results[c]["out"][:S_CORE]
    return out, res


def kernel(**inputs) -> np.ndarray:
    out, _ = run(inputs)
    return out


# revision 16
# speedup vs baseline: 1.0815x; 1.0815x over previous
"""AudioEncoder Trainium2 kernel.

Computes: conv1d(1->64, k=5, stride=2, pad=2) + bias -> ReLU -> per-timestep
linear (64->64) + bias, over audio [4, 480000] f32 -> out [4, 240000, 64] f32.

Strategy (pure data parallel over 8 cores):
  - Each core handles one half of one batch row, padded to S_PAD = 59*2048 =
    120832 output positions (positions >= 120000 are junk, discarded on host).
  - Host pre-gathers the 5 conv tap streams (tap k of output position j is
    x[2j + k - 2], zero-padded) and PERMUTES each into the on-chip column
    order, so every on-chip access is contiguous:
      within each 2048-position block, column c = g*1024 + bi*128 + r
      (g=c//1024, bi=(c//128)%8, r=c%128) holds position u + 16r + 8g + bi.
  - The permuted [5, S_PAD] tap matrix is DMAed twice per 8192-position
    super-block: to SBUF partitions 0-4 (PE row group 0) and 32-36 (row
    group 1), so the four conv matmuls of a chunk run on four disjoint PE
    quadrants (row groups 0/32 x col groups 0/64) with contiguous fp16
    moving operands:
      psc[ 0: 64,   0: 512] <- rg0 cols [0,512)     (g=0, bi=0-3)
      psc[ 0: 64, 512:1024] <- rg1 cols [512,1024)  (g=0, bi=4-7)
      psc[64:128,   0: 512] <- rg0 cols [1024,1536) (g=1, bi=0-3)
      psc[64:128, 512:1024] <- rg1 cols [1536,2048) (g=1, bi=4-7)
    One [128, 1024] PSUM tile (2 banks) holds feats for 2048 positions:
    feats[64g + e, bi*128 + r] = channel e of position u + 16r + 8g + bi.
  - ACT applies conv bias + ReLU in ONE [128, 1024] op (PSUM -> SBUF fp16).
  - Linear: 16 fp16 matmuls per chunk; feats 128-col blocks stationary,
    lin_w.T moving.  A (feats rows 0-63) and B (rows 64-127) matmuls run
    concurrently in different PE row groups and write the two separate PSUM
    banks of one [128, 1024] tile: psl[r, (8g + bi)*64 + p] = out position
    u + 16r + (8g + bi), feature p.  (Same-partition same-bank concurrent
    writes are a HW fault; different banks are safe.)
  - DVE adds the (pre-broadcast) linear bias in ONE [128, 1024] op, writing
    fp16: outt[r, t*64 + p] = out(u + 16r + t, p) -- each SBUF partition is
    ONE 2 KiB contiguous DRAM run (the host upcasts to f32; the 2e-2
    tolerance dwarfs the fp16 output quantization).
  - PSUM budget: conv 2x[128,1024] + linear 2x[128,1024] double-buffered
    = exactly 8 banks.
"""

import numpy as np

import concourse.bacc as bacc
import concourse.bass as bass
import concourse.mybir as mybir
import concourse.tile as tile
from concourse.bass_utils import run_bass_kernel_spmd

B = 4
T = 480000
S_FULL = 240000  # conv output positions per batch row
N_CORES = 8
S_CORE = S_FULL * B // N_CORES  # 120000 positions per core
CHUNK = 2048  # output positions per chunk
S_PAD = ((S_CORE + CHUNK - 1) // CHUNK) * CHUNK  # 120832
SUPER = 8192  # output positions covered per im2col load
E = 64  # conv out channels
P = 64  # linear out features
KS = 5

f16 = mybir.dt.float16
f32 = mybir.dt.float32


def emit(nc: bass.Bass, S: int = S_PAD) -> None:
    """Emit the per-core Tile kernel for S (chunk-aligned) output positions."""
    from contextlib import ExitStack

    im_d = nc.declare_dram_parameter("im5", [KS, S], f16, isOutput=False)
    wc_d = nc.declare_dram_parameter("wc", [40, E], f16, isOutput=False)
    cb_d = nc.declare_dram_parameter("cb", [128, 1], f32, isOutput=False)
    w2_d = nc.declare_dram_parameter("w2", [128, P], f16, isOutput=False)
    b2_d = nc.declare_dram_parameter("b2", [128, 16 * P], f32, isOutput=False)
    out_d = nc.declare_dram_parameter("out", [S, P], f16, isOutput=True)

    RELU = mybir.ActivationFunctionType.Relu
    HALF = CHUNK // 2  # 1024
    assert S % CHUNK == 0

    with tile.TileContext(nc) as tc, ExitStack() as ctx:
        consts = ctx.enter_context(tc.tile_pool(name="consts", bufs=1))
        imp = ctx.enter_context(tc.tile_pool(name="im", bufs=3))
        fpool = ctx.enter_context(tc.tile_pool(name="feats", bufs=3))
        opool = ctx.enter_context(tc.tile_pool(name="outs", bufs=3))
        pc = ctx.enter_context(tc.tile_pool(name="psc", bufs=2, space="PSUM"))
        pl = ctx.enter_context(tc.tile_pool(name="psl", bufs=2, space="PSUM"))

        def load_super(sbase, eng):
            """Load the permuted tap matrix, twice: partitions 0-4 (PE row
            group 0) and 32-36 (row group 1)."""
            scount = min(SUPER, S - sbase)
            t = imp.tile([37, SUPER], f16)
            eng.dma_start(
                out=t[0:KS, 0:scount], in_=im_d[:, sbase : sbase + scount]
            )
            eng.dma_start(
                out=t[32 : 32 + KS, 0:scount],
                in_=im_d[:, sbase : sbase + scount],
            )
            return t

        # super 0 goes FIRST on the sync HWDGE ring so nothing delays the
        # first chunk; the consts queue behind it, then super 1 (still on
        # the sync ring -- its buffer is free, so it can never
        # head-of-line-block the stores).  Later supers are emitted one
        # super AHEAD of use on the gpsimd SWDGE ring: the SWDGE issue
        # waits only on that buffer's release (~4 chunks early with
        # bufs=3), so the data always arrives before the chunk needs it,
        # and a waiting load blocks nothing.
        n_super = (S + SUPER - 1) // SUPER
        # super 0 is loaded chunk-by-chunk so chunk 0 only waits for its own
        # 2x20 KiB slice, not the whole 2x80 KiB super.
        t0 = imp.tile([37, SUPER], f16)
        for cc in range(0, min(SUPER, S), CHUNK):
            nc.sync.dma_start(
                out=t0[0:KS, cc : cc + CHUNK], in_=im_d[:, cc : cc + CHUNK]
            )
            nc.sync.dma_start(
                out=t0[32 : 32 + KS, cc : cc + CHUNK],
                in_=im_d[:, cc : cc + CHUNK],
            )
        tiles = {0: t0}

        # trigger the ACT relu table-set load (~2.7 us) while the first im
        # slices are still in flight, instead of on the first real ACTIVATE.
        scr = consts.tile([1, 2], f32)
        nc.vector.memset(scr[:, :], 0.0)
        nc.scalar.activation(
            out=scr[0:1, 0:1], in_=scr[0:1, 1:2],
            func=mybir.ActivationFunctionType.Relu, scale=1.0,
        )

        wc_sb = consts.tile([40, E], f16)
        nc.sync.dma_start(out=wc_sb[:, :], in_=wc_d[:, :])
        cb_sb = consts.tile([128, 1], f32)
        nc.sync.dma_start(out=cb_sb[:, :], in_=cb_d[:, :])
        w2_sb = consts.tile([128, P], f16)
        nc.sync.dma_start(out=w2_sb[:, :], in_=w2_d[:, :])
        b2_sb = consts.tile([128, 16 * P], f32)
        nc.sync.dma_start(out=b2_sb[:, :], in_=b2_d[:, :])

        if n_super > 1:
            tiles[1] = load_super(SUPER, nc.sync)

        for b in range(0, S, CHUNK):
            si = b // SUPER
            im = tiles[si]

            c0 = b - si * SUPER
            # conv: 4 concurrent matmuls on disjoint PE quadrants, all with
            # contiguous [5, 512] fp16 moving operands.
            psc = pc.tile([128, HALF], f32)  # 2 banks
            for g in (0, 1):  # psum partition half
                q = c0 + 1024 * g
                nc.tensor.matmul(
                    out=psc[E * g : E * g + E, 0:512],
                    lhsT=wc_sb[0:KS, :],
                    rhs=im[0:KS, q : q + 512],
                    start=True, stop=True,
                )
                nc.tensor.matmul(
                    out=psc[E * g : E * g + E, 512:1024],
                    lhsT=wc_sb[32 : 32 + KS, :],
                    rhs=im[32 : 32 + KS, q + 512 : q + 1024],
                    start=True, stop=True,
                )

            feats = fpool.tile([128, HALF], f16)
            nc.scalar.activation(
                out=feats[:, :], in_=psc[:, :], func=RELU,
                bias=cb_sb[:, 0:1], scale=1.0,
            )

            # linear: 8 A/B concurrent pairs; A -> bank 0, B -> bank 1.
            psl = pl.tile([128, HALF], f32)  # 2 banks
            for bi in range(8):
                nc.tensor.matmul(
                    out=psl[:, P * bi : P * bi + P],
                    lhsT=feats[0:E, 128 * bi : 128 * bi + 128],
                    rhs=w2_sb[0:E, :], start=True, stop=True,
                )
                nc.tensor.matmul(
                    out=psl[:, 512 + P * bi : 512 + P * bi + P],
                    lhsT=feats[E : 2 * E, 128 * bi : 128 * bi + 128],
                    rhs=w2_sb[E : 2 * E, :], start=True, stop=True,
                )

            outt = opool.tile([128, HALF], f16)
            nc.vector.tensor_add(outt[:, :], psl[:, :], b2_sb[:, :])

            # s = b + 16r + t ; sbuf col = t*64 + p  (2 KiB run per partition)
            dview = out_d[b : b + CHUNK, :].rearrange("(r t) p -> r t p", t=16)
            sview = outt[:, :].rearrange("r (t p) -> r t p", t=16)
            nc.sync.dma_start(out=dview, in_=sview)

            # prefetch super si+2 right after the first store of super si.
            # Supers 2-3 ride the sync ring: the engine only reaches them
            # after this chunk's store issue, so they cannot contend with
            # the critical super-0/1 loads at t=0 (their buffers are free,
            # so they never head-of-line-block later stores).  Supers >= 4
            # go on the gpsimd ring, where the im pool's slot recycling
            # (bufs=3) gates them to ~8 chunks before first use.
            ci = b // CHUNK
            if ci % 4 == 0 and ci // 4 + 2 < n_super:
                k = ci // 4 + 2
                tiles[k] = load_super(
                    k * SUPER, nc.sync if k <= 3 else nc.gpsimd
                )
                tiles.pop(k - 3, None)


def prep_shared(conv_w, conv_b, lin_w, lin_b):
    """Host-side prep of the (tiny, replicated) parameter tensors."""
    conv_w = np.asarray(conv_w, dtype=np.float32)
    conv_b = np.asarray(conv_b, dtype=np.float32)
    lin_w = np.asarray(lin_w, dtype=np.float32)
    lin_b = np.asarray(lin_b, dtype=np.float32)

    wc5 = conv_w[:, 0, :].T.astype(np.float16)  # [5 taps, 64]
    wc = np.zeros((40, E), dtype=np.float16)
    wc[0:5] = wc5
    wc[32:37] = wc5
    cb = np.ascontiguousarray(
        np.concatenate([conv_b, conv_b]).astype(np.float32)[:, None]
    )  # [128, 1]
    w2 = lin_w.T.astype(np.float16)  # [64e, 64p]
    w2s = np.ascontiguousarray(np.concatenate([w2, w2], axis=0))  # [128, 64]
    b2 = np.ascontiguousarray(
        np.tile(lin_b.astype(np.float32)[None, :], (128, 16))
    )  # [128, 1024]
    return wc, cb, w2s, b2


def prep_inputs(audio_waveform, conv_w, conv_b, lin_w, lin_b):
    """Host-side shard + dtype/layout prep. Returns in_maps for the 8 cores."""
    x = np.asarray(audio_waveform, dtype=np.float32)
    assert x.shape == (B, T)
    # xp[b, 2 + t] = x[b, t]; wide enough for every core's padded window.
    width = 2 * (S_FULL - S_CORE) + 2 * S_PAD + 4
    width = max(width, T + 4)
    xp = np.zeros((B, width), dtype=np.float16)
    xp[:, 2 : 2 + T] = x.astype(np.float16)

    wc, cb, w2s, b2 = prep_shared(conv_w, conv_b, lin_w, lin_b)

    in_maps = []
    for c in range(N_CORES):
        b_i, h = divmod(c, 2)
        s0 = h * S_CORE
        xc = xp[b_i, 2 * s0 : 2 * s0 + 2 * S_PAD + 4]
        # tap k stream (position j -> x[2j + k - 2]), permuted per 2048-block
        # into column order c = g*1024 + bi*128 + r <-> position 16r + 8g + bi.
        rows = np.empty((KS, S_PAD), dtype=np.float16)
        for k in range(KS):
            tap = xc[k : k + 2 * S_PAD : 2]  # [S_PAD]
            rows[k] = (
                tap.reshape(-1, 128, 2, 8).transpose(0, 2, 3, 1).reshape(S_PAD)
            )
        in_maps.append(
            dict(im5=np.ascontiguousarray(rows), wc=wc, cb=cb, w2=w2s, b2=b2)
        )
    return in_maps


_NC_CACHE = None


def get_nc() -> bass.Bass:
    global _NC_CACHE
    if _NC_CACHE is None:
        nc = bacc.Bacc()
        emit(nc)
        # Legalizes TRN2 sync constraints (splits multi-wait instructions),
        # allocates registers, etc. Required before walrus codegen.
        nc.compile()
        _NC_CACHE = nc
    return _NC_CACHE


def run(inputs: dict, trace: bool = False):
    """Run on the 8 cores; returns (full_output, BassKernelResults)."""
    in_maps = prep_inputs(**inputs)
    nc = get_nc()
    res = run_bass_kernel_spmd(nc, in_maps, list(range(N_CORES)), trace=trace)
    out = np.empty((B, S_FULL, P), dtype=np.float32)
    for c in range(N_CORES):
        b_i, h = divmod(c, 2)
        out[b_i, h * S_CORE : (h + 1) * S_CORE, :] = res.results[c]["out"][:S_CORE].astype(np.float32)
    return out, res


def kernel(**inputs) -> np.ndarray:
    out, _ = run(inputs)
    return out


# revision 17
# speedup vs baseline: 1.2644x; 1.1691x over previous
"""AudioEncoder Trainium2 kernel.

Computes: conv1d(1->64, k=5, stride=2, pad=2) + bias -> ReLU -> per-timestep
linear (64->64) + bias, over audio [4, 480000] f32 -> out [4, 240000, 64] f32.

Strategy (pure data parallel over 8 cores):
  - Each core handles one half of one batch row, padded to S_PAD = 59*2048 =
    120832 output positions (positions >= 120000 are junk, discarded on host).
  - Host pre-gathers the 5 conv tap streams (tap k of output position j is
    x[2j + k - 2], zero-padded) and PERMUTES each into the on-chip column
    order, so every on-chip access is contiguous:
      within each 2048-position block, column c = g*1024 + bi*128 + r
      (g=c//1024, bi=(c//128)%8, r=c%128) holds position u + 16r + 8g + bi.
  - The permuted [5, S_PAD] tap matrix is DMAed twice per 8192-position
    super-block: to SBUF partitions 0-4 (PE row group 0) and 32-36 (row
    group 1), so the four conv matmuls of a chunk run on four disjoint PE
    quadrants (row groups 0/32 x col groups 0/64) with contiguous fp16
    moving operands:
      psc[ 0: 64,   0: 512] <- rg0 cols [0,512)     (g=0, bi=0-3)
      psc[ 0: 64, 512:1024] <- rg1 cols [512,1024)  (g=0, bi=4-7)
      psc[64:128,   0: 512] <- rg0 cols [1024,1536) (g=1, bi=0-3)
      psc[64:128, 512:1024] <- rg1 cols [1536,2048) (g=1, bi=4-7)
    One [128, 1024] PSUM tile (2 banks) holds feats for 2048 positions:
    feats[64g + e, bi*128 + r] = channel e of position u + 16r + 8g + bi.
  - ACT applies conv bias + ReLU in ONE [128, 1024] op (PSUM -> SBUF fp16).
  - Linear: 16 fp16 matmuls per chunk; feats 128-col blocks stationary,
    lin_w.T moving.  A (feats rows 0-63) and B (rows 64-127) matmuls run
    concurrently in different PE row groups and write the two separate PSUM
    banks of one [128, 1024] tile: psl[r, (8g + bi)*64 + p] = out position
    u + 16r + (8g + bi), feature p.  (Same-partition same-bank concurrent
    writes are a HW fault; different banks are safe.)
  - DVE adds the (pre-broadcast) linear bias in ONE [128, 1024] op, writing
    fp16: outt[r, t*64 + p] = out(u + 16r + t, p) -- each SBUF partition is
    ONE 2 KiB contiguous DRAM run (the host upcasts to f32; the 2e-2
    tolerance dwarfs the fp16 output quantization).
  - PSUM budget: conv 2x[128,1024] + linear 2x[128,1024] double-buffered
    = exactly 8 banks.
"""

import numpy as np

import concourse.bacc as bacc
import concourse.bass as bass
import concourse.mybir as mybir
import concourse.tile as tile
from concourse.bass_utils import run_bass_kernel_spmd

B = 4
T = 480000
S_FULL = 240000  # conv output positions per batch row
N_CORES = 8
S_CORE = S_FULL * B // N_CORES  # 120000 positions per core
CHUNK = 2048  # output positions per chunk
S_PAD = ((S_CORE + CHUNK - 1) // CHUNK) * CHUNK  # 120832
SUPER = 8192  # output positions covered per im2col load
E = 64  # conv out channels
P = 64  # linear out features
KS = 5

f16 = mybir.dt.float16
f32 = mybir.dt.float32


def emit(nc: bass.Bass, S: int = S_PAD) -> None:
    """Emit the per-core Tile kernel for S (chunk-aligned) output positions."""
    from contextlib import ExitStack

    im_d = nc.declare_dram_parameter("im5", [KS, S], f16, isOutput=False)
    wc_d = nc.declare_dram_parameter("wc", [40, E], f16, isOutput=False)
    cb_d = nc.declare_dram_parameter("cb", [128, 1], f32, isOutput=False)
    w2_d = nc.declare_dram_parameter("w2", [128, P], f16, isOutput=False)
    b2_d = nc.declare_dram_parameter("b2", [128, 16 * P], f32, isOutput=False)
    out_d = nc.declare_dram_parameter("out", [S, P], f16, isOutput=True)

    RELU = mybir.ActivationFunctionType.Relu
    HALF = CHUNK // 2  # 1024
    assert S % CHUNK == 0

    with tile.TileContext(nc) as tc, ExitStack() as ctx:
        consts = ctx.enter_context(tc.tile_pool(name="consts", bufs=1))
        imp = ctx.enter_context(tc.tile_pool(name="im", bufs=3))
        fpool = ctx.enter_context(tc.tile_pool(name="feats", bufs=3))
        opool = ctx.enter_context(tc.tile_pool(name="outs", bufs=10))
        pc = ctx.enter_context(tc.tile_pool(name="psc", bufs=2, space="PSUM"))
        pl = ctx.enter_context(tc.tile_pool(name="psl", bufs=2, space="PSUM"))

        def load_super(sbase, eng):
            """Load the permuted tap matrix, twice: partitions 0-4 (PE row
            group 0) and 32-36 (row group 1)."""
            scount = min(SUPER, S - sbase)
            t = imp.tile([37, SUPER], f16)
            eng.dma_start(
                out=t[0:KS, 0:scount], in_=im_d[:, sbase : sbase + scount]
            )
            eng.dma_start(
                out=t[32 : 32 + KS, 0:scount],
                in_=im_d[:, sbase : sbase + scount],
            )
            return t

        # super 0 goes FIRST on the sync HWDGE ring so nothing delays the
        # first chunk; the consts queue behind it, then super 1 (still on
        # the sync ring -- its buffer is free, so it can never
        # head-of-line-block the stores).  Later supers are emitted one
        # super AHEAD of use on the gpsimd SWDGE ring: the SWDGE issue
        # waits only on that buffer's release (~4 chunks early with
        # bufs=3), so the data always arrives before the chunk needs it,
        # and a waiting load blocks nothing.
        n_super = (S + SUPER - 1) // SUPER

        # trigger the ACT relu table-set load (~2.7 us) while the first
        # loads are still in flight, instead of on the first real ACTIVATE.
        scr = consts.tile([1, 2], f32)
        nc.vector.memset(scr[:, :], 0.0)
        nc.scalar.activation(
            out=scr[0:1, 0:1], in_=scr[0:1, 1:2],
            func=mybir.ActivationFunctionType.Relu, scale=1.0,
        )

        # startup order on the sync ring: the tiny consts the first chunk
        # needs, then chunk 0's im slice, then the rest of super 0 chunk by
        # chunk (so chunk 0 only waits for its own 2x20 KiB slice, not the
        # whole 2x80 KiB super), then b2 (first needed by chunk 0's DVE).
        wc_sb = consts.tile([40, E], f16)
        nc.sync.dma_start(out=wc_sb[:, :], in_=wc_d[:, :])
        cb_sb = consts.tile([128, 1], f32)
        nc.sync.dma_start(out=cb_sb[:, :], in_=cb_d[:, :])
        w2_sb = consts.tile([128, P], f16)
        nc.sync.dma_start(out=w2_sb[:, :], in_=w2_d[:, :])

        t0 = imp.tile([37, SUPER], f16)
        for cc in range(0, min(SUPER, S), CHUNK):
            nc.sync.dma_start(
                out=t0[0:KS, cc : cc + CHUNK], in_=im_d[:, cc : cc + CHUNK]
            )
            nc.sync.dma_start(
                out=t0[32 : 32 + KS, cc : cc + CHUNK],
                in_=im_d[:, cc : cc + CHUNK],
            )
            if cc == 0:
                b2_sb = consts.tile([128, 16 * P], f32)
                nc.sync.dma_start(out=b2_sb[:, :], in_=b2_d[:, :])
        tiles = {0: t0}

        if n_super > 1:
            tiles[1] = load_super(SUPER, nc.sync)

        for b in range(0, S, CHUNK):
            si = b // SUPER
            im = tiles[si]

            c0 = b - si * SUPER
            # conv: 4 concurrent matmuls on disjoint PE quadrants, all with
            # contiguous [5, 512] fp16 moving operands.
            psc = pc.tile([128, HALF], f32)  # 2 banks
            for g in (0, 1):  # psum partition half
                q = c0 + 1024 * g
                nc.tensor.matmul(
                    out=psc[E * g : E * g + E, 0:512],
                    lhsT=wc_sb[0:KS, :],
                    rhs=im[0:KS, q : q + 512],
                    start=True, stop=True,
                )
                nc.tensor.matmul(
                    out=psc[E * g : E * g + E, 512:1024],
                    lhsT=wc_sb[32 : 32 + KS, :],
                    rhs=im[32 : 32 + KS, q + 512 : q + 1024],
                    start=True, stop=True,
                )

            feats = fpool.tile([128, HALF], f16)
            nc.scalar.activation(
                out=feats[:, :], in_=psc[:, :], func=RELU,
                bias=cb_sb[:, 0:1], scale=1.0,
            )

            # linear: 8 A/B concurrent pairs; A -> bank 0, B -> bank 1.
            psl = pl.tile([128, HALF], f32)  # 2 banks
            for bi in range(8):
                nc.tensor.matmul(
                    out=psl[:, P * bi : P * bi + P],
                    lhsT=feats[0:E, 128 * bi : 128 * bi + 128],
                    rhs=w2_sb[0:E, :], start=True, stop=True,
                )
                nc.tensor.matmul(
                    out=psl[:, 512 + P * bi : 512 + P * bi + P],
                    lhsT=feats[E : 2 * E, 128 * bi : 128 * bi + 128],
                    rhs=w2_sb[E : 2 * E, :], start=True, stop=True,
                )

            outt = opool.tile([128, HALF], f16)
            nc.vector.tensor_add(outt[:, :], psl[:, :], b2_sb[:, :])

            # s = b + 16r + t ; sbuf col = t*64 + p  (2 KiB run per partition)
            dview = out_d[b : b + CHUNK, :].rearrange("(r t) p -> r t p", t=16)
            sview = outt[:, :].rearrange("r (t p) -> r t p", t=16)
            nc.sync.dma_start(out=dview, in_=sview)

            # prefetch super si+2 right after the first store of super si.
            # Supers 2-3 ride the sync ring: the engine only reaches them
            # after this chunk's store issue, so they cannot contend with
            # the critical super-0/1 loads at t=0 (their buffers are free,
            # so they never head-of-line-block later stores).  Supers >= 4
            # go on the gpsimd ring, where the im pool's slot recycling
            # (bufs=3) gates them to ~8 chunks before first use.
            ci = b // CHUNK
            if ci % 4 == 0 and ci // 4 + 2 < n_super:
                k = ci // 4 + 2
                tiles[k] = load_super(
                    k * SUPER, nc.sync if k <= 3 else nc.gpsimd
                )
                tiles.pop(k - 3, None)


def prep_shared(conv_w, conv_b, lin_w, lin_b):
    """Host-side prep of the (tiny, replicated) parameter tensors."""
    conv_w = np.asarray(conv_w, dtype=np.float32)
    conv_b = np.asarray(conv_b, dtype=np.float32)
    lin_w = np.asarray(lin_w, dtype=np.float32)
    lin_b = np.asarray(lin_b, dtype=np.float32)

    wc5 = conv_w[:, 0, :].T.astype(np.float16)  # [5 taps, 64]
    wc = np.zeros((40, E), dtype=np.float16)
    wc[0:5] = wc5
    wc[32:37] = wc5
    cb = np.ascontiguousarray(
        np.concatenate([conv_b, conv_b]).astype(np.float32)[:, None]
    )  # [128, 1]
    w2 = lin_w.T.astype(np.float16)  # [64e, 64p]
    w2s = np.ascontiguousarray(np.concatenate([w2, w2], axis=0))  # [128, 64]
    b2 = np.ascontiguousarray(
        np.tile(lin_b.astype(np.float32)[None, :], (128, 16))
    )  # [128, 1024]
    return wc, cb, w2s, b2


def prep_inputs(audio_waveform, conv_w, conv_b, lin_w, lin_b):
    """Host-side shard + dtype/layout prep. Returns in_maps for the 8 cores."""
    x = np.asarray(audio_waveform, dtype=np.float32)
    assert x.shape == (B, T)
    # xp[b, 2 + t] = x[b, t]; wide enough for every core's padded window.
    width = 2 * (S_FULL - S_CORE) + 2 * S_PAD + 4
    width = max(width, T + 4)
    xp = np.zeros((B, width), dtype=np.float16)
    xp[:, 2 : 2 + T] = x.astype(np.float16)

    wc, cb, w2s, b2 = prep_shared(conv_w, conv_b, lin_w, lin_b)

    in_maps = []
    for c in range(N_CORES):
        b_i, h = divmod(c, 2)
        s0 = h * S_CORE
        xc = xp[b_i, 2 * s0 : 2 * s0 + 2 * S_PAD + 4]
        # tap k stream (position j -> x[2j + k - 2]), permuted per 2048-block
        # into column order c = g*1024 + bi*128 + r <-> position 16r + 8g + bi.
        rows = np.empty((KS, S_PAD), dtype=np.float16)
        for k in range(KS):
            tap = xc[k : k + 2 * S_PAD : 2]  # [S_PAD]
            rows[k] = (
                tap.reshape(-1, 128, 2, 8).transpose(0, 2, 3, 1).reshape(S_PAD)
            )
        in_maps.append(
            dict(im5=np.ascontiguousarray(rows), wc=wc, cb=cb, w2=w2s, b2=b2)
        )
    return in_maps


_NC_CACHE = None


def get_nc() -> bass.Bass:
    global _NC_CACHE
    if _NC_CACHE is None:
        nc = bacc.Bacc()
        emit(nc)
        # Legalizes TRN2 sync constraints (splits multi-wait instructions),
        # allocates registers, etc. Required before walrus codegen.
        nc.compile()
        _NC_CACHE = nc
    return _NC_CACHE


def run(inputs: dict, trace: bool = False):
    """Run on the 8 cores; returns (full_output, BassKernelResults)."""
    in_maps = prep_inputs(**inputs)
    nc = get_nc()
    res = run_bass_kernel_spmd(nc, in_maps, list(range(N_CORES)), trace=trace)
    out = np.empty((B, S_FULL, P), dtype=np.float32)
    for c in range(N_CORES):
        b_i, h = divmod(c, 2)
        out[b_i, h * S_CORE : (h + 1) * S_CORE, :] = res.results[c]["out"][:S_CORE].astype(np.float32)
    return out, res


def kernel(**inputs) -> np.ndarray:
    out, _ = run(inputs)
    return out


# revision 18
# speedup vs baseline: 1.3207x; 1.0445x over previous
"""AudioEncoder Trainium2 kernel.

Computes: conv1d(1->64, k=5, stride=2, pad=2) + bias -> ReLU -> per-timestep
linear (64->64) + bias, over audio [4, 480000] f32 -> out [4, 240000, 64] f32.

Strategy (pure data parallel over 8 cores):
  - Each core handles one half of one batch row, padded to S_PAD = 59*2048 =
    120832 output positions (positions >= 120000 are junk, discarded on host).
  - Host pre-gathers the 5 conv tap streams (tap k of output position j is
    x[2j + k - 2], zero-padded) and PERMUTES each into the on-chip column
    order, so every on-chip access is contiguous:
      within each 2048-position block, column c = g*1024 + bi*128 + r
      (g=c//1024, bi=(c//128)%8, r=c%128) holds position u + 16r + 8g + bi.
  - The permuted [5, S_PAD] tap matrix is DMAed twice per 8192-position
    super-block: to SBUF partitions 0-4 (PE row group 0) and 32-36 (row
    group 1), so the four conv matmuls of a chunk run on four disjoint PE
    quadrants (row groups 0/32 x col groups 0/64) with contiguous fp16
    moving operands:
      psc[ 0: 64,   0: 512] <- rg0 cols [0,512)     (g=0, bi=0-3)
      psc[ 0: 64, 512:1024] <- rg1 cols [512,1024)  (g=0, bi=4-7)
      psc[64:128,   0: 512] <- rg0 cols [1024,1536) (g=1, bi=0-3)
      psc[64:128, 512:1024] <- rg1 cols [1536,2048) (g=1, bi=4-7)
    One [128, 1024] PSUM tile (2 banks) holds feats for 2048 positions:
    feats[64g + e, bi*128 + r] = channel e of position u + 16r + 8g + bi.
  - ACT applies conv bias + ReLU in ONE [128, 1024] op (PSUM -> SBUF fp16).
  - Linear: 16 fp16 matmuls per chunk; feats 128-col blocks stationary,
    lin_w.T moving.  A (feats rows 0-63) and B (rows 64-127) matmuls run
    concurrently in different PE row groups and write the two separate PSUM
    banks of one [128, 1024] tile: psl[r, (8g + bi)*64 + p] = out position
    u + 16r + (8g + bi), feature p.  (Same-partition same-bank concurrent
    writes are a HW fault; different banks are safe.)
  - DVE adds the (pre-broadcast) linear bias in ONE [128, 1024] op, writing
    fp16: outt[r, t*64 + p] = out(u + 16r + t, p) -- each SBUF partition is
    ONE 2 KiB contiguous DRAM run (the host upcasts to f32; the 2e-2
    tolerance dwarfs the fp16 output quantization).
  - PSUM budget: conv 2x[128,1024] + linear 2x[128,1024] double-buffered
    = exactly 8 banks.
"""

import numpy as np

import concourse.bacc as bacc
import concourse.bass as bass
import concourse.mybir as mybir
import concourse.tile as tile
from concourse.bass_utils import run_bass_kernel_spmd

B = 4
T = 480000
S_FULL = 240000  # conv output positions per batch row
N_CORES = 8
S_CORE = S_FULL * B // N_CORES  # 120000 positions per core
CHUNK = 2048  # output positions per chunk
S_PAD = ((S_CORE + CHUNK - 1) // CHUNK) * CHUNK  # 120832
SUPER = 8192  # output positions covered per im2col load
E = 64  # conv out channels
P = 64  # linear out features
KS = 5

f16 = mybir.dt.float16
f32 = mybir.dt.float32


def emit(nc: bass.Bass, S: int = S_PAD) -> None:
    """Emit the per-core Tile kernel for S (chunk-aligned) output positions."""
    from contextlib import ExitStack

    im_d = nc.declare_dram_parameter("im5", [KS, S], f16, isOutput=False)
    wc_d = nc.declare_dram_parameter("wc", [40, E], f16, isOutput=False)
    cb_d = nc.declare_dram_parameter("cb", [128, 1], f32, isOutput=False)
    w2_d = nc.declare_dram_parameter("w2", [128, P], f16, isOutput=False)
    b2_d = nc.declare_dram_parameter("b2", [128, 16 * P], f32, isOutput=False)
    out_d = nc.declare_dram_parameter("out", [S, P], f16, isOutput=True)

    RELU = mybir.ActivationFunctionType.Relu
    HALF = CHUNK // 2  # 1024
    assert S % CHUNK == 0

    with tile.TileContext(nc) as tc, ExitStack() as ctx:
        consts = ctx.enter_context(tc.tile_pool(name="consts", bufs=1))
        imp = ctx.enter_context(tc.tile_pool(name="im", bufs=4))
        fpool = ctx.enter_context(tc.tile_pool(name="feats", bufs=3))
        opool = ctx.enter_context(tc.tile_pool(name="outs", bufs=5))
        pc = ctx.enter_context(tc.tile_pool(name="psc", bufs=2, space="PSUM"))
        pl = ctx.enter_context(tc.tile_pool(name="psl", bufs=2, space="PSUM"))

        def load_super(sbase, eng):
            """Load the permuted tap matrix, twice: partitions 0-4 (PE row
            group 0) and 32-36 (row group 1)."""
            scount = min(SUPER, S - sbase)
            t = imp.tile([37, SUPER], f16)
            eng.dma_start(
                out=t[0:KS, 0:scount], in_=im_d[:, sbase : sbase + scount]
            )
            eng.dma_start(
                out=t[32 : 32 + KS, 0:scount],
                in_=im_d[:, sbase : sbase + scount],
            )
            return t

        # super 0 goes FIRST on the sync HWDGE ring so nothing delays the
        # first chunk; the consts queue behind it, then super 1 (still on
        # the sync ring -- its buffer is free, so it can never
        # head-of-line-block the stores).  Later supers are emitted one
        # super AHEAD of use on the gpsimd SWDGE ring: the SWDGE issue
        # waits only on that buffer's release (~4 chunks early with
        # bufs=3), so the data always arrives before the chunk needs it,
        # and a waiting load blocks nothing.
        n_super = (S + SUPER - 1) // SUPER

        # trigger the ACT relu table-set load (~2.7 us) while the first
        # loads are still in flight, instead of on the first real ACTIVATE.
        scr = consts.tile([1, 2], f32)
        nc.vector.memset(scr[:, :], 0.0)
        nc.scalar.activation(
            out=scr[0:1, 0:1], in_=scr[0:1, 1:2],
            func=mybir.ActivationFunctionType.Relu, scale=1.0,
        )

        # startup order on the sync ring: the tiny consts the first chunk
        # needs, then chunk 0's im slice, then the rest of super 0 chunk by
        # chunk (so chunk 0 only waits for its own 2x20 KiB slice, not the
        # whole 2x80 KiB super), then b2 (first needed by chunk 0's DVE).
        wc_sb = consts.tile([40, E], f16)
        nc.sync.dma_start(out=wc_sb[:, :], in_=wc_d[:, :])
        cb_sb = consts.tile([128, 1], f32)
        nc.sync.dma_start(out=cb_sb[:, :], in_=cb_d[:, :])
        w2_sb = consts.tile([128, P], f16)
        nc.sync.dma_start(out=w2_sb[:, :], in_=w2_d[:, :])

        t0 = imp.tile([37, SUPER], f16)
        for cc in range(0, min(SUPER, S), CHUNK):
            nc.sync.dma_start(
                out=t0[0:KS, cc : cc + CHUNK], in_=im_d[:, cc : cc + CHUNK]
            )
            nc.sync.dma_start(
                out=t0[32 : 32 + KS, cc : cc + CHUNK],
                in_=im_d[:, cc : cc + CHUNK],
            )
            if cc == 0:
                b2_sb = consts.tile([128, 16 * P], f32)
                nc.sync.dma_start(out=b2_sb[:, :], in_=b2_d[:, :])
        tiles = {0: t0}

        if n_super > 1:
            tiles[1] = load_super(SUPER, nc.sync)

        for b in range(0, S, CHUNK):
            si = b // SUPER
            im = tiles[si]

            c0 = b - si * SUPER
            # conv: 4 concurrent matmuls on disjoint PE quadrants, all with
            # contiguous [5, 512] fp16 moving operands.
            psc = pc.tile([128, HALF], f32)  # 2 banks
            for g in (0, 1):  # psum partition half
                q = c0 + 1024 * g
                nc.tensor.matmul(
                    out=psc[E * g : E * g + E, 0:512],
                    lhsT=wc_sb[0:KS, :],
                    rhs=im[0:KS, q : q + 512],
                    start=True, stop=True,
                )
                nc.tensor.matmul(
                    out=psc[E * g : E * g + E, 512:1024],
                    lhsT=wc_sb[32 : 32 + KS, :],
                    rhs=im[32 : 32 + KS, q + 512 : q + 1024],
                    start=True, stop=True,
                )

            feats = fpool.tile([128, HALF], f16)
            nc.scalar.activation(
                out=feats[:, :], in_=psc[:, :], func=RELU,
                bias=cb_sb[:, 0:1], scale=1.0,
            )

            # linear: 8 A/B concurrent pairs; A -> bank 0, B -> bank 1.
            psl = pl.tile([128, HALF], f32)  # 2 banks
            for bi in range(8):
                nc.tensor.matmul(
                    out=psl[:, P * bi : P * bi + P],
                    lhsT=feats[0:E, 128 * bi : 128 * bi + 128],
                    rhs=w2_sb[0:E, :], start=True, stop=True,
                )
                nc.tensor.matmul(
                    out=psl[:, 512 + P * bi : 512 + P * bi + P],
                    lhsT=feats[E : 2 * E, 128 * bi : 128 * bi + 128],
                    rhs=w2_sb[E : 2 * E, :], start=True, stop=True,
                )

            # accumulate TWO chunks into one [128, 2048] out tile and store
            # them with a single DMA: halves the store-DMA count, so the
            # DMA-completion sem lanes recycle half as fast (their recycle
            # waits were stalling the im loads and forcing an early
            # mid-kernel Tile sem-reset barrier).
            ci = b // CHUNK
            par = ci % 2
            if par == 0:
                outt = opool.tile([128, 2 * HALF], f16)
            nc.vector.tensor_add(
                outt[:, par * HALF : par * HALF + HALF], psl[:, :], b2_sb[:, :]
            )
            if par == 1:
                # s = (b-CHUNK) + 2048h + 16r + t ; sbuf col = h*1024+t*64+p
                dview = out_d[b - CHUNK : b + CHUNK, :].rearrange(
                    "(h r t) p -> r h t p", h=2, t=16
                )
                sview = outt[:, :].rearrange("r (h t p) -> r h t p", h=2, t=16)
                nc.sync.dma_start(out=dview, in_=sview)
            elif b + CHUNK >= S:  # unpaired final chunk
                dview = out_d[b : b + CHUNK, :].rearrange(
                    "(r t) p -> r t p", t=16
                )
                sview = outt[:, 0:HALF].rearrange("r (t p) -> r t p", t=16)
                nc.sync.dma_start(out=dview, in_=sview)

            # prefetch im supers.  Supers 2-3 ride the sync ring right
            # after chunk 0: the engine only reaches them after the first
            # store issue, so they cannot contend with the critical
            # super-0/1 loads at t=0 (their buffers are free, so they
            # never head-of-line-block later stores).  Supers >= 4 go on
            # the gpsimd ring three supers ahead of use, where the im
            # pool's slot recycling (bufs=4) gates them to ~13 chunks
            # before first use.
            if ci == 0:
                for k in (2, 3):
                    if k < n_super:
                        tiles[k] = load_super(k * SUPER, nc.sync)
            elif ci % 4 == 0:
                k = ci // 4 + 3
                if 4 <= k < n_super:
                    tiles[k] = load_super(k * SUPER, nc.gpsimd)
                tiles.pop(k - 5, None)


def prep_shared(conv_w, conv_b, lin_w, lin_b):
    """Host-side prep of the (tiny, replicated) parameter tensors."""
    conv_w = np.asarray(conv_w, dtype=np.float32)
    conv_b = np.asarray(conv_b, dtype=np.float32)
    lin_w = np.asarray(lin_w, dtype=np.float32)
    lin_b = np.asarray(lin_b, dtype=np.float32)

    wc5 = conv_w[:, 0, :].T.astype(np.float16)  # [5 taps, 64]
    wc = np.zeros((40, E), dtype=np.float16)
    wc[0:5] = wc5
    wc[32:37] = wc5
    cb = np.ascontiguousarray(
        np.concatenate([conv_b, conv_b]).astype(np.float32)[:, None]
    )  # [128, 1]
    w2 = lin_w.T.astype(np.float16)  # [64e, 64p]
    w2s = np.ascontiguousarray(np.concatenate([w2, w2], axis=0))  # [128, 64]
    b2 = np.ascontiguousarray(
        np.tile(lin_b.astype(np.float32)[None, :], (128, 16))
    )  # [128, 1024]
    return wc, cb, w2s, b2


def prep_inputs(audio_waveform, conv_w, conv_b, lin_w, lin_b):
    """Host-side shard + dtype/layout prep. Returns in_maps for the 8 cores."""
    x = np.asarray(audio_waveform, dtype=np.float32)
    assert x.shape == (B, T)
    # xp[b, 2 + t] = x[b, t]; wide enough for every core's padded window.
    width = 2 * (S_FULL - S_CORE) + 2 * S_PAD + 4
    width = max(width, T + 4)
    xp = np.zeros((B, width), dtype=np.float16)
    xp[:, 2 : 2 + T] = x.astype(np.float16)

    wc, cb, w2s, b2 = prep_shared(conv_w, conv_b, lin_w, lin_b)

    in_maps = []
    for c in range(N_CORES):
        b_i, h = divmod(c, 2)
        s0 = h * S_CORE
        xc = xp[b_i, 2 * s0 : 2 * s0 + 2 * S_PAD + 4]
        # tap k stream (position j -> x[2j + k - 2]), permuted per 2048-block
        # into column order c = g*1024 + bi*128 + r <-> position 16r + 8g + bi.
        rows = np.empty((KS, S_PAD), dtype=np.float16)
        for k in range(KS):
            tap = xc[k : k + 2 * S_PAD : 2]  # [S_PAD]
            rows[k] = (
                tap.reshape(-1, 128, 2, 8).transpose(0, 2, 3, 1).reshape(S_PAD)
            )
        in_maps.append(
            dict(im5=np.ascontiguousarray(rows), wc=wc, cb=cb, w2=w2s, b2=b2)
        )
    return in_maps


_NC_CACHE = None


def get_nc() -> bass.Bass:
    global _NC_CACHE
    if _NC_CACHE is None:
        nc = bacc.Bacc()
        emit(nc)
        # Legalizes TRN2 sync constraints (splits multi-wait instructions),
        # allocates registers, etc. Required before walrus codegen.
        nc.compile()
        _NC_CACHE = nc
    return _NC_CACHE


def run(inputs: dict, trace: bool = False):
    """Run on the 8 cores; returns (full_output, BassKernelResults)."""
    in_maps = prep_inputs(**inputs)
    nc = get_nc()
    res = run_bass_kernel_spmd(nc, in_maps, list(range(N_CORES)), trace=trace)
    out = np.empty((B, S_FULL, P), dtype=np.float32)
    for c in range(N_CORES):
        b_i, h = divmod(c, 2)
        out[b_i, h * S_CORE : (h + 1) * S_CORE, :] = res.results[c]["out"][:S_CORE].astype(np.float32)
    return out, res


def kernel(**inputs) -> np.ndarray:
    out, _ = run(inputs)
    return out
